# revision 14
# baseline (speedup 1.0000x reference)
"""Distributed causal attention (RoPE, QKV/out projections) on 8 TRN2 NeuronCores.

Sharding: batch x head-group. 2 batches x 4 head-groups of 4 heads; core
c = b*4 + g handles batch b, heads [4g, 4g+4).  Each core:
  - reads only ITS batch's transposed activations xT [D, S] (4MB, half of
    the head-only-TP baseline - the start is HBM bandwidth-bound)
  - computes qT/kT for its 4 heads as two 128-partition "pairs", applies
    RoPE (pair-swap via a +-1 permutation matmul; combine split between
    DVE and GPSIMD so the vector engine isn't the bottleneck)
  - computes v for all 4 heads in one 256-wide matmul per 128-row s-tile
    (x-tile stationary shared), augmented with ones-columns so PV also
    produces the softmax denominator
  - flash-style causal attention per head-pair with scores transposed
    [sk, sq]: quadrant-packed score matmuls + -240-staircase mask
    matmuls, ACT exp, then PV with the ones-column denominator trick
  - row-sharded output projection (256 local e-dims, 2 accumulating
    matmuls per 128-col e-tile) -> partial output [D, S] for its batch
Host sums each batch's 4 partials and transposes back to [B, S, D].

Performance structure:
  - ~26 junk warmup matmuls at t=0 keep the PE busy during the DMA head
    so the HAM clock gate un-throttles (1.2->2.4 GHz) ~3.4us in instead
    of ~25us in
  - x loads stream per 512-col chunk across the sync/gpsimd queues; wq/
    wk (sync/gpsimd) and wv (scalar queue) land first so P(0) projection
    starts ~4us in
  - filler pieces (projection/output-projection work) are interleaved
    into the attention tile loops; a filler for chunk c only rides an
    attention pass of chunk j < c (its products are consumed one chunk
    later), avoiding engine-order deadlocks
  - ACT runs exp-only mid-kernel; normalization copies ride ACT in the
    pass-boundary bubble, reciprocal+muls on DVE at 2x (bf16 dense),
    rope combine partially on GPSIMD
  - PSUM: 4 banks score ring (bufs=2) + 2 banks PV accumulator + 2-bank
    work ring shared by projection/output pieces
fp8 was tried in an earlier session and REVERTED: random-sign dot
products give no sqrt(N) error averaging -> ~4e-2 output error, over the
2e-2 gate. bf16 everywhere.
"""

import numpy as np
import ml_dtypes

import concourse.mybir as mybir
from concourse import bacc
import concourse.tile as tile
from concourse.bass import ts, ds

B, S, D, H, HD = 2, 2048, 1024, 16, 64
NCORES = 8
HL = 4                      # heads per core
EL = HL * HD                # local e-dims per core = 256
DCH = D // 128              # 8 contraction chunks
NCH = S // 512              # 4 column chunks (one batch)
NKT = S // 128              # 16 k-tiles
THETA = 10000.0
BF = mybir.dt.bfloat16
F32 = mybir.dt.float32
EXPFN = mybir.ActivationFunctionType.Exp
CPYFN = mybir.ActivationFunctionType.Copy
NWARM = 8                   # junk matmuls to trip the HAM clock gate early

_nc_cache = {}


def build_nc(debug=False):
    key = bool(debug)
    if key in _nc_cache:
        return _nc_cache[key]
    nc = bacc.Bacc("TRN2", target_bir_lowering=False, debug=debug, num_devices=NCORES)

    xT_d = nc.dram_tensor("xT", [D, S], BF, kind="ExternalInput")
    cos_d = nc.dram_tensor("cosT", [128, S], BF, kind="ExternalInput")
    sin_d = nc.dram_tensor("sinT", [128, S], BF, kind="ExternalInput")
    wq_d = nc.dram_tensor("wqT", [D, EL], BF, kind="ExternalInput")
    wk_d = nc.dram_tensor("wkT", [D, EL], BF, kind="ExternalInput")
    perm_d = nc.dram_tensor("permT", [128, 128], BF, kind="ExternalInput")
    wv_d = nc.dram_tensor("wvT", [D, EL], BF, kind="ExternalInput")
    wo_d = nc.dram_tensor("woT", [EL, D], BF, kind="ExternalInput")
    mi_d = nc.dram_tensor("maskI", [128, 64], BF, kind="ExternalInput")
    su_d = nc.dram_tensor("stepU", [128, 2, 128], BF, kind="ExternalInput")
    out_d = nc.dram_tensor("out", [D, S], BF, kind="ExternalOutput")

    with tile.TileContext(nc) as tc:
        with (
            tc.tile_pool(name="sb", bufs=1) as sb,
            tc.tile_pool(name="work", bufs=2) as work,
            tc.tile_pool(name="ps", bufs=1, space="PSUM") as ps,
        ):
            # ---- persistent SBUF tensors ----
            xts = sb.tile([128, DCH, S], BF)       # x transposed, d on partitions
            wqs = sb.tile([128, DCH, EL], BF)
            wks = sb.tile([128, DCH, EL], BF)
            perms = sb.tile([128, 128], BF)
            wvs = sb.tile([128, DCH, EL], BF)
            wos = sb.tile([128, 2, D], BF)         # [p, pair, e'] of woT
            coss = sb.tile([128, S], BF)
            sins = sb.tile([128, S], BF)
            qt = sb.tile([128, 2, S], BF)          # [hd-in-pair, pair, s]
            kt = sb.tile([128, 2, S], BF)
            vaug = sb.tile([128, NKT, 260], BF)    # per k-tile: 2 pairs x [v|1|v|1]
            attnT = sb.tile([128, 2, S], BF)       # normalized attn out per pair
            mi64s = sb.tile([128, 64], BF)         # -240 * [p%64 == m]
            steps = sb.tile([128, 2, 128], BF)     # [(p%64) + 64s > q]
            junks = sb.tile([128, 512], BF)

            # ---- PE warmup: one long accumulation group of junk matmuls
            # (no intermediate events -> dense PE activity) so the HAM
            # clock gate flips to 2.4 GHz ~3.4us in, bridging the DMA head ----
            nc.vector.memset(junks[:], 0.0)
            junkp = ps.tile([128, 2, 512], F32, tag="sc", bufs=2, name="junk")
            for i in range(NWARM):
                nc.tensor.matmul(junkp[:, 0, :], junks[:, 0:128], junks[:],
                                 start=(i == 0), stop=(i == NWARM - 1))

            # ---- input DMAs.  wq/wk/wv first (unblock P(0)), x streams
            # per chunk with chunk 0 first; stores later ride sync. ----
            def xcols(c, k, eng):
                eng.dma_start(
                    out=xts[:, k, ds(c * 512, 512)],
                    in_=xT_d[ts(k, 128), ds(c * 512, 512)],
                )

            nc.sync.dma_start(out=wqs[:], in_=wq_d[:, :].rearrange("(k p) e -> p k e", p=128))
            nc.gpsimd.dma_start(out=wks[:], in_=wk_d[:, :].rearrange("(k p) e -> p k e", p=128))
            for k in range(DCH):
                xcols(0, k, (nc.sync, nc.gpsimd, nc.scalar)[k % 3])
            nc.scalar.dma_start(out=wvs[:], in_=wv_d[:, :].rearrange("(k p) e -> p k e", p=128))
            nc.sync.dma_start(out=perms[:], in_=perm_d[:, :])
            nc.sync.dma_start(out=mi64s[:], in_=mi_d[:, :])
            nc.sync.dma_start(out=steps[:], in_=su_d[:, :, :])
            # cos/sin stream per 512-col chunk just ahead of that chunk's
            # rope; x chunks c>=1 follow so chunk c lands just before its
            # projection pieces pop
            nc.gpsimd.dma_start(out=coss[:, 0:512], in_=cos_d[:, 0:512])
            nc.gpsimd.dma_start(out=sins[:, 0:512], in_=sin_d[:, 0:512])
            for c in range(1, NCH):
                for k in range(DCH):
                    xcols(c, k, (nc.sync, nc.gpsimd)[k % 2])
                cc = ds(c * 512, 512)
                nc.gpsimd.dma_start(out=coss[:, cc], in_=cos_d[:, cc])
                nc.gpsimd.dma_start(out=sins[:, cc], in_=sin_d[:, cc])
            nc.gpsimd.dma_start(out=wos[:], in_=wo_d[:, :].rearrange("(g p) e -> p g e", p=128))

            # ones columns for the PV denominator rows (full memset also
            # marks the tensor initialized for strided writes)
            nc.gpsimd.memset(vaug[:], 1.0)
            scratch1 = sb.tile([128, 1], F32)
            nc.vector.memset(scratch1[:], 0.0)
            nc.scalar.activation(scratch1[:], scratch1[:], EXPFN)  # preload exp table

            # ---- QKV projection pieces for one 512-col chunk c ----
            def proj_pieces(c):
                cs = ds(c * 512, 512)

                def qk(wtile, rot, nm, p):
                    st = {}
                    ep = ds(128 * p, 128)

                    def mms_a():
                        pp = ps.tile([128, 512], F32, tag="work", bufs=2,
                                     name=f"pp{nm}{c}{p}")
                        for k in range(4):
                            nc.tensor.matmul(
                                pp[:], wtile[:, k, ep], xts[:, k, cs],
                                start=(k == 0), stop=False,
                            )
                        st["pp"] = pp

                    def mms_b():
                        pp = st["pp"]
                        for k in range(4, DCH):
                            nc.tensor.matmul(
                                pp[:], wtile[:, k, ep], xts[:, k, cs],
                                start=False, stop=(k == DCH - 1),
                            )
                        raw = work.tile([128, 512], BF, tag="rawt", bufs=2,
                                        name="raw")
                        nc.vector.tensor_copy(raw[:], pp[:])
                        st["raw"] = raw

                    def ropep():
                        raw = st["raw"]
                        sp2 = ps.tile([128, 512], F32, tag="work", bufs=2,
                                      name="sp2")
                        nc.tensor.matmul(sp2[:], perms[:], raw[:],
                                         start=True, stop=True)
                        rtmp = work.tile([128, 512], BF, tag="ropetmp",
                                         bufs=2, name="rtmp")
                        nc.vector.tensor_mul(rot[:, p, cs], raw[:], coss[:, cs])
                        nc.vector.tensor_mul(rtmp[:], sp2[:], sins[:, cs])
                        nc.vector.tensor_add(rot[:, p, cs], rot[:, p, cs], rtmp[:])

                    return [mms_a, mms_b, ropep]

                def vtile(st4):
                    def go():
                        t = c * 4 + st4
                        vp = ps.tile([128, 512], F32, tag="work", bufs=2,
                                     name=f"vp{t}")
                        for k in range(DCH):
                            nc.tensor.matmul(
                                vp[:, 0:256], xts[:, k, ds(t * 128, 128)], wvs[:, k, :],
                                start=(k == 0), stop=(k == DCH - 1),
                            )
                        dst = vaug[:, t, :].rearrange("p (g y) -> p g y", g=4)[:, :, 0:64]
                        src = vp[:, 0:256].rearrange("p (g y) -> p g y", g=4)
                        nc.vector.tensor_copy(dst, src)
                    return go

                q0 = qk(wqs, qt, "q", 0)
                q1 = qk(wqs, qt, "q", 1)
                k0 = qk(wks, kt, "k", 0)
                k1 = qk(wks, kt, "k", 1)
                v = [vtile(s) for s in range(4)]
                # each rope (which waits on a DVE round-trip) is issued two
                # pieces after its matmuls so the PE never bubbles on it
                return [q0[0], q0[1], q1[0], q0[2], q1[1], k0[0], q1[2],
                        k0[1], k1[0], k0[2], k1[1], v[0], k1[2], v[1], v[2], v[3]]

            # ---- output projection pieces for one chunk (needs both
            # attention passes of that chunk complete) ----
            def oproj_pieces(j, tail=False):
                oc = ds(j * 512, 512)
                ost = work.tile([128, 8, 512], BF, tag="ostage", bufs=2, name="ost")

                def piece(e):
                    def go():
                        op = ps.tile([128, 512], F32, tag="work", bufs=2, name="op")
                        nc.tensor.matmul(op[:], wos[:, 0, ts(e, 128)], attnT[:, 0, oc],
                                         start=True, stop=False)
                        nc.tensor.matmul(op[:], wos[:, 1, ts(e, 128)], attnT[:, 1, oc],
                                         start=False, stop=True)
                        act = (e % 2 == 1) if tail else False
                        if act:
                            nc.scalar.copy(ost[:, e, :], op[:])
                        else:
                            nc.vector.tensor_copy(ost[:, e, :], op[:])
                    return go

                def store():
                    nc.sync.dma_start(
                        out=out_d.rearrange("(ec p) s -> p ec s", p=128)[:, :, ts(j, 512)],
                        in_=ost[:],
                    )
                return [piece(e) for e in range(8)] + [store]

            # ---- causal attention for one (chunk j, head-pair p) ----
            def attn_pass(j, p, fillers=(), last=False):
                fillers = list(fillers)
                ntk = 4 * (j + 1)
                pv = ps.tile([65, 1024], F32, tag="pv", bufs=1, name=f"pv{j}{p}")[:]
                qc0 = j * 512

                def pv_mms(t, pt):
                    off = max(0, 128 * (t - 4 * j))
                    w = 512 - off
                    nc.tensor.matmul(
                        pv[0:65, ds(off, w)], vaug[:, t, ds(130 * p, 65)],
                        pt[:, 0, off:512],
                        start=(t == 0), stop=(t == ntk - 1),
                    )
                    nc.tensor.matmul(
                        pv[0:65, ds(512 + off, w)], vaug[:, t, ds(130 * p + 65, 65)],
                        pt[:, 1, off:512],
                        start=(t == 0), stop=(t == ntk - 1),
                    )

                prev = None  # software-pipeline: PV(t-1) issues after scores(t)
                for t in range(ntk):
                    off = max(0, 128 * (t - 4 * j))
                    w = 512 - off
                    diag = t >= 4 * j
                    sc = ps.tile([128, 2, 512], F32, tag="sc", bufs=2, name="sc")
                    pt = work.tile([128, 2, 512], BF, tag="ptile", bufs=3, name="pt")
                    kc = t * 128
                    # 4 concurrent quadrant matmuls (head-in-pair x k-subtile)
                    for h in range(2):
                        for s2 in range(2):
                            nc.tensor.matmul(
                                sc[64 * s2:64 * s2 + 64, h, off:512],
                                kt[64 * h:64 * h + 64, p, ds(kc + 64 * s2, 64)],
                                qt[64 * h:64 * h + 64, p, ds(qc0 + off, w)],
                                start=True, stop=not diag,
                            )
                    if diag:
                        # accumulate -240 onto sub-diagonal (q<k) positions;
                        # exp then flushes them to ~0 without a mask multiply
                        for h in range(2):
                            for s2 in range(2):
                                nc.tensor.matmul(
                                    sc[64 * s2:64 * s2 + 64, h, off:off + 128],
                                    mi64s[64 * h:64 * h + 64, :],
                                    steps[64 * h:64 * h + 64, s2, :],
                                    start=False, stop=True,
                                )
                    nc.scalar.activation(
                        pt[:, :, off:512], sc[:, :, off:512], EXPFN, scale=0.125,
                    )
                    # spread filler pieces over remaining slots; one before
                    # PV(t-1) to cover its wait on exp(t-1)
                    quota = (len(fillers) + ntk - t - 1) // (ntk - t)
                    if quota and fillers:
                        fillers.pop(0)()
                        quota -= 1
                    if prev is not None:
                        pv_mms(*prev)
                    for _ in range(quota):
                        if fillers:
                            fillers.pop(0)()
                    prev = (t, pt)
                while fillers:
                    fillers.pop(0)()
                pv_mms(*prev)
                # normalize by the denominator row (pv row 64).  Copies ride
                # ACT (pass-boundary bubble), reciprocal + muls on DVE with
                # a bf16 multiplier so the muls hit the 2x perf mode.
                oc = ds(j * 512, 512)
                if last:
                    # half-width pipelined chain to cut the tail latency
                    for h in range(2):
                        hs = ds(h * 512, 512)
                        lbuf = work.tile([1, 512], F32, tag="lbufh", bufs=2, name="lbuf")
                        rbuf = work.tile([1, 512], F32, tag="rbufh", bufs=2, name="rbuf")
                        rbh = work.tile([1, 512], BF, tag="rbhh", bufs=2, name="rbh")
                        pvc = work.tile([64, 512], BF, tag="pvch", bufs=2, name="pvc")
                        rb = work.tile([64, 512], BF, tag="rbh2", bufs=2, name="rb")
                        nc.scalar.copy(lbuf[:], pv[64:65, hs])
                        nc.vector.tensor_copy(pvc[:], pv[0:64, hs])
                        nc.vector.reciprocal_approx_fast(rbuf[:], lbuf[:])
                        nc.vector.tensor_copy(rbh[:], rbuf[:])
                        nc.gpsimd.partition_broadcast(rb[:], rbh[:], channels=64)
                        nc.vector.tensor_mul(
                            attnT[64 * h:64 * h + 64, p, oc], pvc[:], rb[:],
                        )
                else:
                    lbuf = work.tile([1, 1024], F32, tag="lbuf", bufs=2, name="lbuf")
                    rbuf = work.tile([1, 1024], F32, tag="rbuf", bufs=2, name="rbuf")
                    rbh = work.tile([1, 1024], BF, tag="rbh", bufs=2, name="rbh")
                    pvc = work.tile([64, 1024], BF, tag="pvc", bufs=2, name="pvc")
                    rb = work.tile([64, 1024], BF, tag="rb", bufs=2, name="rb")
                    nc.scalar.copy(lbuf[:], pv[64:65, :])
                    nc.vector.tensor_copy(pvc[:], pv[0:64, :])
                    nc.vector.reciprocal_approx_fast(rbuf[:], lbuf[:])
                    nc.vector.tensor_copy(rbh[:], rbuf[:])
                    nc.gpsimd.partition_broadcast(rb[:], rbh[:], channels=64)
                    nc.vector.tensor_mul(attnT[0:64, p, oc], pvc[:, 0:512], rb[:, 0:512])
                    nc.vector.tensor_mul(attnT[64:128, p, oc], pvc[:, 512:1024], rb[:, 512:1024])

            def oproj_streamed(j):
                # split the final store so DMA overlaps the last copies
                oc = ds(j * 512, 512)
                ost = work.tile([128, 8, 512], BF, tag="ostage", bufs=2, name="ost")
                od = out_d.rearrange("(ec p) s -> p ec s", p=128)
                for e in range(8):
                    op = ps.tile([128, 512], F32, tag="work", bufs=2, name="op")
                    nc.tensor.matmul(op[:], wos[:, 0, ts(e, 128)], attnT[:, 0, oc],
                                     start=True, stop=False)
                    nc.tensor.matmul(op[:], wos[:, 1, ts(e, 128)], attnT[:, 1, oc],
                                     start=False, stop=True)
                    if e % 2 == 1:
                        nc.scalar.copy(ost[:, e, :], op[:])
                    else:
                        nc.vector.tensor_copy(ost[:, e, :], op[:])
                    if e == 3:
                        nc.sync.dma_start(out=od[:, 0:4, ts(j, 512)], in_=ost[:, 0:4, :])
                nc.sync.dma_start(out=od[:, 4:8, ts(j, 512)], in_=ost[:, 4:8, :])

            # ---- schedule ----
            # P(c) may only ride an attention pass of chunk j < c (products
            # consumed a chunk later); O(c) only after both passes of c.
            # Chunk order 0,2,3,1: big chunks mid-kernel where filler is
            # plentiful, medium chunk 1 last with O-fillers, then O(1).
            P = proj_pieces
            O = oproj_pieces
            for f in P(0) + P(1):
                f()
            p2 = P(2)
            attn_pass(1, 0, p2[:8])
            attn_pass(1, 1, p2[8:])
            p3 = P(3)
            attn_pass(2, 0, p3[:8])
            attn_pass(2, 1, p3[8:])
            attn_pass(3, 0, O(1))
            attn_pass(3, 1, O(2))
            o3 = O(3)
            attn_pass(0, 0, o3[:5])
            attn_pass(0, 1, o3[5:], last=True)
            oproj_streamed(0)

    nc.compile()
    _nc_cache[key] = nc
    return nc


def make_in_maps(x, token_positions, wq, wk, wv, wo):
    bf = ml_dtypes.bfloat16
    x = np.asarray(x, np.float32)
    pos = np.asarray(token_positions, np.float64)
    inv_freq = THETA ** (-(2.0 * np.arange(HD // 2, dtype=np.float64) / HD))
    ang = pos[:, None] * inv_freq[None, :]          # [S, 32]
    cos = np.cos(ang).astype(np.float32)
    sin = np.sin(ang).astype(np.float32)
    p = np.arange(128)
    idx = (p % HD) // 2
    cosT = np.ascontiguousarray(cos[:, idx].T).astype(bf)             # [128, S]
    sinT = np.ascontiguousarray(sin[:, idx].T).astype(bf)

    wq = np.asarray(wq, np.float32)
    wk = np.asarray(wk, np.float32)
    wv = np.asarray(wv, np.float32)
    wo = np.asarray(wo, np.float32)

    permT = np.zeros((128, 128), np.float32)
    me = np.arange(0, 128, 2)
    permT[me + 1, me] = -1.0      # swapped[even m] = -raw[m+1]
    permT[me, me + 1] = 1.0       # swapped[odd m]  = +raw[m-1]
    permT = permT.astype(bf)

    pp_ = np.arange(128)
    mm_ = np.arange(64)
    maskI = (-240.0 * ((pp_[:, None] % 64) == mm_[None, :])).astype(np.float32).astype(bf)
    qq_ = np.arange(128)
    stepU = np.stack(
        [(pp_[:, None] % 64) + 64 * s > qq_[None, :] for s in range(2)], axis=1
    ).astype(np.float32).astype(bf)   # [128, 2, 128]

    xTb = [np.ascontiguousarray(x[b].T).astype(bf) for b in range(B)]

    in_maps = []
    for c in range(NCORES):
        b, g = c // 4, c % 4
        rows = slice(g * EL, (g + 1) * EL)
        in_maps.append({
            "xT": xTb[b],
            "cosT": cosT,
            "sinT": sinT,
            "wqT": np.ascontiguousarray(wq[rows, :].T).astype(bf),
            "wkT": np.ascontiguousarray(wk[rows, :].T).astype(bf),
            "permT": permT,
            "wvT": np.ascontiguousarray(wv[rows, :].T).astype(bf),
            "woT": np.ascontiguousarray(wo[:, rows].T).astype(bf),
            "maskI": maskI,
            "stepU": stepU,
        })
    return in_maps


def unshard(results):
    acc = np.zeros((B, D, S), np.float32)
    for c, r in enumerate(results):
        acc[c // 4] += np.asarray(r["out"], np.float32)
    return np.ascontiguousarray(acc.transpose(0, 2, 1))


def kernel(x, token_positions, wq, wk, wv, wo):
    from concourse.bass_utils import run_bass_kernel_spmd

    nc = build_nc(debug=False)
    in_maps = make_in_maps(x, token_positions, wq, wk, wv, wo)
    res = run_bass_kernel_spmd(nc, in_maps, core_ids=list(range(NCORES)))
    return unshard(res.results)


if __name__ == "__main__":
    # smoke test with random data
    rng = np.random.default_rng(0)
    x = rng.standard_normal((B, S, D), dtype=np.float32)
    tp = np.arange(S, dtype=np.int32)
    ws = [rng.standard_normal((D, D), dtype=np.float32) * 0.02 for _ in range(4)]
    out = kernel(x, tp, *ws)
    print(out.shape, out.dtype)


# revision 19
# speedup vs baseline: 1.2399x; 1.2399x over previous
"""Distributed causal attention (RoPE, QKV/out projections) on 8 TRN2 NeuronCores.

Sharding: batch x head-group. 2 batches x 4 head-groups of 4 heads; core
c = b*4 + g handles batch b, heads [4g, 4g+4).  Each core:
  - reads only ITS batch's transposed activations xT [D, S] (4MB, half of
    the head-only-TP baseline - the start is HBM bandwidth-bound)
  - computes qT/kT for its 4 heads as two 128-partition "pairs", applies
    RoPE (pair-swap via a +-1 permutation matmul; combine split between
    DVE and GPSIMD so the vector engine isn't the bottleneck)
  - computes v for all 4 heads in one 256-wide matmul per 128-row s-tile
    (x-tile stationary shared), augmented with ones-columns so PV also
    produces the softmax denominator
  - flash-style causal attention per head-pair with scores transposed
    [sk, sq]: quadrant-packed score matmuls + -240-staircase mask
    matmuls, ACT exp, then PV with the ones-column denominator trick
  - row-sharded output projection (256 local e-dims, 2 accumulating
    matmuls per 128-col e-tile) -> partial output [D, S] for its batch
Host sums each batch's 4 partials and transposes back to [B, S, D].

Performance structure:
  - ~26 junk warmup matmuls at t=0 keep the PE busy during the DMA head
    so the HAM clock gate un-throttles (1.2->2.4 GHz) ~3.4us in instead
    of ~25us in
  - x loads stream per 512-col chunk across the sync/gpsimd queues; wq/
    wk (sync/gpsimd) and wv (scalar queue) land first so P(0) projection
    starts ~4us in
  - filler pieces (projection/output-projection work) are interleaved
    into the attention tile loops; a filler for chunk c only rides an
    attention pass of chunk j < c (its products are consumed one chunk
    later), avoiding engine-order deadlocks
  - ACT runs exp-only mid-kernel; normalization copies ride ACT in the
    pass-boundary bubble, reciprocal+muls on DVE at 2x (bf16 dense),
    rope combine partially on GPSIMD
  - PSUM: 4 banks score ring (bufs=2) + 2 banks PV accumulator + 2-bank
    work ring shared by projection/output pieces
fp8 was tried in an earlier session and REVERTED: random-sign dot
products give no sqrt(N) error averaging -> ~4e-2 output error, over the
2e-2 gate. bf16 everywhere.
"""

import numpy as np
import ml_dtypes

import concourse.mybir as mybir
from concourse import bacc
import concourse.tile as tile
from concourse.bass import ts, ds

B, S, D, H, HD = 2, 2048, 1024, 16, 64
NCORES = 8
HL = 4                      # heads per core
EL = HL * HD                # local e-dims per core = 256
DCH = D // 128              # 8 contraction chunks
NCH = S // 512              # 4 column chunks (one batch)
NKT = S // 128              # 16 k-tiles
THETA = 10000.0
BF = mybir.dt.bfloat16
F32 = mybir.dt.float32
EXPFN = mybir.ActivationFunctionType.Exp
CPYFN = mybir.ActivationFunctionType.Copy
NWARM = 8                   # junk matmuls to trip the HAM clock gate early

_nc_cache = {}


def build_nc(debug=False):
    key = bool(debug)
    if key in _nc_cache:
        return _nc_cache[key]
    nc = bacc.Bacc("TRN2", target_bir_lowering=False, debug=debug, num_devices=NCORES)

    xT_d = nc.dram_tensor("xT", [D, S], BF, kind="ExternalInput")
    cos_d = nc.dram_tensor("cosT", [128, S], BF, kind="ExternalInput")
    sin_d = nc.dram_tensor("sinT", [128, S], BF, kind="ExternalInput")
    wq_d = nc.dram_tensor("wqT", [D, EL], BF, kind="ExternalInput")
    wk_d = nc.dram_tensor("wkT", [D, EL], BF, kind="ExternalInput")
    perm_d = nc.dram_tensor("permT", [128, 128], BF, kind="ExternalInput")
    wv_d = nc.dram_tensor("wvT", [D, EL], BF, kind="ExternalInput")
    wo_d = nc.dram_tensor("woT", [EL, D], BF, kind="ExternalInput")
    mi_d = nc.dram_tensor("maskI", [128, 64], BF, kind="ExternalInput")
    su_d = nc.dram_tensor("stepU", [128, 2, 128], BF, kind="ExternalInput")
    out_d = nc.dram_tensor("out", [D, S], BF, kind="ExternalOutput")
    # chunk-3 output projection is split by head-pair into two partials
    # (summed on the host) so half of it can run inside the last pass
    o3a_d = nc.dram_tensor("o3a", [D, 512], BF, kind="ExternalOutput")
    o3b_d = nc.dram_tensor("o3b", [D, 512], BF, kind="ExternalOutput")

    with tile.TileContext(nc) as tc:
        with (
            tc.tile_pool(name="sb", bufs=1) as sb,
            tc.tile_pool(name="work", bufs=2) as work,
            tc.tile_pool(name="ps", bufs=1, space="PSUM") as ps,
        ):
            # ---- persistent SBUF tensors ----
            xts = sb.tile([128, DCH, S], BF)       # x transposed, d on partitions
            wqs = sb.tile([128, DCH, EL], BF)
            wks = sb.tile([128, DCH, EL], BF)
            perms = sb.tile([128, 128], BF)
            wvs = sb.tile([128, DCH, EL], BF)
            wos = sb.tile([128, 2, D], BF)         # [p, pair, e'] of woT
            coss = sb.tile([128, S], BF)
            sins = sb.tile([128, S], BF)
            qt = sb.tile([128, 2, S], BF)          # [hd-in-pair, pair, s]
            kt = sb.tile([128, 2, S], BF)
            vaug = sb.tile([128, NKT, 260], BF)    # per k-tile: 2 pairs x [v|1|v|1]
            attnT = sb.tile([128, 2, S], BF)       # normalized attn out per pair
            mi64s = sb.tile([128, 64], BF)         # -240 * [p%64 == m]
            steps = sb.tile([128, 2, 128], BF)     # [(p%64) + 64s > q]
            junks = sb.tile([128, 512], BF)

            # ---- PE warmup: one long accumulation group of junk matmuls
            # (no intermediate events -> dense PE activity) so the HAM
            # clock gate flips to 2.4 GHz ~3.4us in, bridging the DMA head ----
            nc.vector.memset(junks[:], 0.0)
            junkp = ps.tile([128, 2, 512], F32, tag="sc", bufs=2, name="junk")
            for i in range(NWARM):
                nc.tensor.matmul(junkp[:, 0, :], junks[:, 0:128], junks[:],
                                 start=(i == 0), stop=(i == NWARM - 1))

            # ---- input DMAs.  wq/wk/wv first (unblock P(0)), x streams
            # per chunk with chunk 0 first; stores later ride sync. ----
            def xcols(c, k, eng):
                eng.dma_start(
                    out=xts[:, k, ds(c * 512, 512)],
                    in_=xT_d[ts(k, 128), ds(c * 512, 512)],
                )

            nc.sync.dma_start(out=wqs[:], in_=wq_d[:, :].rearrange("(k p) e -> p k e", p=128))
            nc.gpsimd.dma_start(out=wks[:], in_=wk_d[:, :].rearrange("(k p) e -> p k e", p=128))
            for k in range(DCH):
                xcols(0, k, (nc.sync, nc.gpsimd, nc.scalar)[k % 3])
            nc.scalar.dma_start(out=wvs[:], in_=wv_d[:, :].rearrange("(k p) e -> p k e", p=128))
            nc.sync.dma_start(out=perms[:], in_=perm_d[:, :])
            nc.sync.dma_start(out=mi64s[:], in_=mi_d[:, :])
            nc.sync.dma_start(out=steps[:], in_=su_d[:, :, :])
            # cos/sin stream per 512-col chunk just ahead of that chunk's
            # rope; x chunks c>=1 follow so chunk c lands just before its
            # projection pieces pop
            nc.gpsimd.dma_start(out=coss[:, 0:512], in_=cos_d[:, 0:512])
            nc.gpsimd.dma_start(out=sins[:, 0:512], in_=sin_d[:, 0:512])
            for c in range(1, NCH):
                for k in range(DCH):
                    xcols(c, k, (nc.sync, nc.gpsimd)[k % 2])
                cc = ds(c * 512, 512)
                nc.gpsimd.dma_start(out=coss[:, cc], in_=cos_d[:, cc])
                nc.gpsimd.dma_start(out=sins[:, cc], in_=sin_d[:, cc])
            nc.gpsimd.dma_start(out=wos[:], in_=wo_d[:, :].rearrange("(g p) e -> p g e", p=128))

            # ones columns for the PV denominator rows (full memset also
            # marks the tensor initialized for strided writes)
            nc.gpsimd.memset(vaug[:], 1.0)
            scratch1 = sb.tile([128, 1], F32)
            nc.vector.memset(scratch1[:], 0.0)
            nc.scalar.activation(scratch1[:], scratch1[:], EXPFN)  # preload exp table

            # ---- QKV projection pieces for one 512-col chunk c ----
            def proj_pieces(c):
                cs = ds(c * 512, 512)

                def qk(wtile, rot, nm, p):
                    st = {}
                    ep = ds(128 * p, 128)

                    def mms_a():
                        pp = ps.tile([128, 512], F32, tag="work", bufs=2,
                                     name=f"pp{nm}{c}{p}")
                        for k in range(4):
                            nc.tensor.matmul(
                                pp[:], wtile[:, k, ep], xts[:, k, cs],
                                start=(k == 0), stop=False,
                            )
                        st["pp"] = pp

                    def mms_b():
                        pp = st["pp"]
                        for k in range(4, DCH):
                            nc.tensor.matmul(
                                pp[:], wtile[:, k, ep], xts[:, k, cs],
                                start=False, stop=(k == DCH - 1),
                            )
                        raw = work.tile([128, 512], BF, tag="rawt", bufs=2,
                                        name="raw")
                        nc.vector.tensor_copy(raw[:], pp[:])
                        st["raw"] = raw

                    def ropep():
                        raw = st["raw"]
                        sp2 = ps.tile([128, 512], F32, tag="work", bufs=2,
                                      name="sp2")
                        nc.tensor.matmul(sp2[:], perms[:], raw[:],
                                         start=True, stop=True)
                        rtmp = work.tile([128, 512], BF, tag="ropetmp",
                                         bufs=2, name="rtmp")
                        # rtmp first: frees the sp2 psum ring slot after one
                        # DVE op so the next rope's perm matmul isn't gated
                        # on this whole chain
                        nc.vector.tensor_mul(rtmp[:], sp2[:], sins[:, cs])
                        nc.vector.tensor_mul(rot[:, p, cs], raw[:], coss[:, cs])
                        nc.vector.tensor_add(rot[:, p, cs], rot[:, p, cs], rtmp[:])

                    return [mms_a, mms_b, ropep]

                def vtile(st4):
                    def go():
                        t = c * 4 + st4
                        vp = ps.tile([128, 512], F32, tag="work", bufs=2,
                                     name=f"vp{t}")
                        for k in range(DCH):
                            nc.tensor.matmul(
                                vp[:, 0:256], xts[:, k, ds(t * 128, 128)], wvs[:, k, :],
                                start=(k == 0), stop=(k == DCH - 1),
                            )
                        dst = vaug[:, t, :].rearrange("p (g y) -> p g y", g=4)[:, :, 0:64]
                        src = vp[:, 0:256].rearrange("p (g y) -> p g y", g=4)
                        nc.vector.tensor_copy(dst, src)
                    return go

                q0 = qk(wqs, qt, "q", 0)
                q1 = qk(wqs, qt, "q", 1)
                k0 = qk(wks, kt, "k", 0)
                k1 = qk(wks, kt, "k", 1)
                v = [vtile(s) for s in range(4)]
                # each rope (which waits on a DVE round-trip) is issued two
                # pieces after its matmuls so the PE never bubbles on it
                return [q0[0], q0[1], q1[0], q0[2], q1[1], k0[0], q1[2],
                        k0[1], k1[0], k0[2], k1[1], v[0], k1[2], v[1], v[2], v[3]]

            # ---- output projection pieces for one chunk (needs both
            # attention passes of that chunk complete) ----
            def oproj_pieces(j, tail=False):
                oc = ds(j * 512, 512)
                ost = work.tile([128, 8, 512], BF, tag="ostage", bufs=2, name="ost")

                def piece(e):
                    def go():
                        op = ps.tile([128, 512], F32, tag="work", bufs=2, name="op")
                        nc.tensor.matmul(op[:], wos[:, 0, ts(e, 128)], attnT[:, 0, oc],
                                         start=True, stop=False)
                        nc.tensor.matmul(op[:], wos[:, 1, ts(e, 128)], attnT[:, 1, oc],
                                         start=False, stop=True)
                        act = (e % 2 == 1) if tail else False
                        if act:
                            nc.scalar.copy(ost[:, e, :], op[:])
                        else:
                            nc.vector.tensor_copy(ost[:, e, :], op[:])
                    return go

                def store():
                    nc.sync.dma_start(
                        out=out_d.rearrange("(ec p) s -> p ec s", p=128)[:, :, ts(j, 512)],
                        in_=ost[:],
                    )
                return [piece(e) for e in range(8)] + [store]

            # ---- causal attention for one (chunk j, head-pair p) ----
            def attn_pass(j, p, fillers=(), last=False):
                fillers = list(fillers)
                ntk = 4 * (j + 1)
                pv = ps.tile([65, 1024], F32, tag="pv", bufs=1, name=f"pv{j}{p}")[:]
                qc0 = j * 512

                def pv_mms(t, pt):
                    off = max(0, 128 * (t - 4 * j))
                    w = 512 - off
                    nc.tensor.matmul(
                        pv[0:65, ds(off, w)], vaug[:, t, ds(130 * p, 65)],
                        pt[:, 0, off:512],
                        start=(t == 0), stop=(t == ntk - 1),
                    )
                    nc.tensor.matmul(
                        pv[0:65, ds(512 + off, w)], vaug[:, t, ds(130 * p + 65, 65)],
                        pt[:, 1, off:512],
                        start=(t == 0), stop=(t == ntk - 1),
                    )

                prev = None  # software-pipeline: PV(t-1) issues after scores(t)
                for t in range(ntk):
                    off = max(0, 128 * (t - 4 * j))
                    w = 512 - off
                    diag = t >= 4 * j
                    sc = ps.tile([128, 2, 512], F32, tag="sc", bufs=2, name="sc")
                    pt = work.tile([128, 2, 512], BF, tag="ptile", bufs=3, name="pt")
                    kc = t * 128
                    # 4 concurrent quadrant matmuls (head-in-pair x k-subtile)
                    for h in range(2):
                        for s2 in range(2):
                            nc.tensor.matmul(
                                sc[64 * s2:64 * s2 + 64, h, off:512],
                                kt[64 * h:64 * h + 64, p, ds(kc + 64 * s2, 64)],
                                qt[64 * h:64 * h + 64, p, ds(qc0 + off, w)],
                                start=True, stop=not diag,
                            )
                    if diag:
                        # accumulate -240 onto sub-diagonal (q<k) positions;
                        # exp then flushes them to ~0 without a mask multiply
                        for h in range(2):
                            for s2 in range(2):
                                nc.tensor.matmul(
                                    sc[64 * s2:64 * s2 + 64, h, off:off + 128],
                                    mi64s[64 * h:64 * h + 64, :],
                                    steps[64 * h:64 * h + 64, s2, :],
                                    start=False, stop=True,
                                )
                    nc.scalar.activation(
                        pt[:, :, off:512], sc[:, :, off:512], EXPFN, scale=0.125,
                    )
                    # spread filler pieces over remaining slots; one before
                    # PV(t-1) to cover its wait on exp(t-1)
                    quota = (len(fillers) + ntk - t - 1) // (ntk - t)
                    if quota and fillers:
                        fillers.pop(0)()
                        quota -= 1
                    if prev is not None:
                        pv_mms(*prev)
                    for _ in range(quota):
                        if fillers:
                            fillers.pop(0)()
                    prev = (t, pt)
                while fillers:
                    fillers.pop(0)()
                pv_mms(*prev)
                # normalize by the denominator row (pv row 64).  Copies ride
                # ACT (pass-boundary bubble), reciprocal + muls on DVE with
                # a bf16 multiplier so the muls hit the 2x perf mode.
                oc = ds(j * 512, 512)
                if last:
                    # half-width pipelined chain to cut the tail latency
                    for h in range(2):
                        hs = ds(h * 512, 512)
                        lbuf = work.tile([1, 512], F32, tag="lbufh", bufs=2, name="lbuf")
                        rbuf = work.tile([1, 512], F32, tag="rbufh", bufs=2, name="rbuf")
                        rbh = work.tile([1, 512], BF, tag="rbhh", bufs=2, name="rbh")
                        pvc = work.tile([64, 512], BF, tag="pvch", bufs=2, name="pvc")
                        rb = work.tile([64, 512], BF, tag="rbh2", bufs=2, name="rb")
                        nc.scalar.copy(lbuf[:], pv[64:65, hs])
                        nc.vector.tensor_copy(pvc[:], pv[0:64, hs])
                        nc.vector.reciprocal_approx_fast(rbuf[:], lbuf[:])
                        nc.vector.tensor_copy(rbh[:], rbuf[:])
                        nc.gpsimd.partition_broadcast(rb[:], rbh[:], channels=64)
                        nc.vector.tensor_mul(
                            attnT[64 * h:64 * h + 64, p, oc], pvc[:], rb[:],
                        )
                else:
                    lbuf = work.tile([1, 1024], F32, tag="lbuf", bufs=2, name="lbuf")
                    rbuf = work.tile([1, 1024], F32, tag="rbuf", bufs=2, name="rbuf")
                    rbh = work.tile([1, 1024], BF, tag="rbh", bufs=2, name="rbh")
                    pvc = work.tile([64, 1024], BF, tag="pvc", bufs=2, name="pvc")
                    rb = work.tile([64, 1024], BF, tag="rb", bufs=2, name="rb")
                    nc.scalar.copy(lbuf[:], pv[64:65, :])
                    nc.vector.tensor_copy(pvc[:], pv[0:64, :])
                    nc.vector.reciprocal_approx_fast(rbuf[:], lbuf[:])
                    nc.vector.tensor_copy(rbh[:], rbuf[:])
                    nc.gpsimd.partition_broadcast(rb[:], rbh[:], channels=64)
                    nc.vector.tensor_mul(attnT[0:64, p, oc], pvc[:, 0:512], rb[:, 0:512])
                    nc.vector.tensor_mul(attnT[64:128, p, oc], pvc[:, 512:1024], rb[:, 512:1024])

            def opart_pieces(p, dst_d):
                # single-pair partial output projection of chunk 3 (the
                # last chunk) -> its own DRAM partial, summed on the host
                oc = ds(3 * 512, 512)
                ost = work.tile([128, 8, 512], BF, tag="ostage", bufs=2, name="ost")

                def piece(e):
                    def go():
                        op = ps.tile([128, 512], F32, tag="work", bufs=2, name="op")
                        nc.tensor.matmul(op[:], wos[:, p, ts(e, 128)], attnT[:, p, oc],
                                         start=True, stop=True)
                        nc.vector.tensor_copy(ost[:, e, :], op[:])
                    return go

                def store():
                    nc.sync.dma_start(
                        out=dst_d.rearrange("(ec p) s -> p ec s", p=128),
                        in_=ost[:],
                    )
                return [piece(e) for e in range(8)] + [store]

            def opart_streamed(p, dst_d):
                # pair-1 chunk-3 partial: the true kernel tail, with split
                # stores and alternating copy engines
                oc = ds(3 * 512, 512)
                ost = work.tile([128, 8, 512], BF, tag="ostage", bufs=2, name="ost")
                od = dst_d.rearrange("(ec p) s -> p ec s", p=128)
                for e in range(8):
                    op = ps.tile([128, 512], F32, tag="work", bufs=2, name="op")
                    nc.tensor.matmul(op[:], wos[:, p, ts(e, 128)], attnT[:, p, oc],
                                     start=True, stop=True)
                    if e % 2 == 1:
                        nc.scalar.copy(ost[:, e, :], op[:])
                    else:
                        nc.vector.tensor_copy(ost[:, e, :], op[:])
                    if e == 3:
                        nc.sync.dma_start(out=od[:, 0:4, :], in_=ost[:, 0:4, :])
                nc.sync.dma_start(out=od[:, 4:8, :], in_=ost[:, 4:8, :])

            # ---- schedule ----
            # P(c) may only ride an attention pass of chunk j < c (products
            # consumed a chunk later); O(c) only after both passes of c.
            # Chunk order 0,2,3,1: big chunks mid-kernel where filler is
            # plentiful, medium chunk 1 last with O-fillers, then O(1).
            P = proj_pieces
            O = oproj_pieces
            for f in P(0):
                f()
            p1 = P(1)
            attn_pass(0, 0, p1[:8])
            attn_pass(0, 1, p1[8:])
            p2 = P(2)
            attn_pass(1, 0, p2[:8])
            attn_pass(1, 1, p2[8:])
            p3 = P(3)
            o0 = O(0)
            attn_pass(2, 0, p3[:8] + o0[:4])
            attn_pass(2, 1, p3[8:] + o0[4:])
            attn_pass(3, 0, O(1))
            attn_pass(3, 1, O(2) + opart_pieces(0, o3a_d), last=True)
            opart_streamed(1, o3b_d)

    nc.compile()
    _nc_cache[key] = nc
    return nc


def make_in_maps(x, token_positions, wq, wk, wv, wo):
    bf = ml_dtypes.bfloat16
    x = np.asarray(x, np.float32)
    pos = np.asarray(token_positions, np.float64)
    inv_freq = THETA ** (-(2.0 * np.arange(HD // 2, dtype=np.float64) / HD))
    ang = pos[:, None] * inv_freq[None, :]          # [S, 32]
    cos = np.cos(ang).astype(np.float32)
    sin = np.sin(ang).astype(np.float32)
    p = np.arange(128)
    idx = (p % HD) // 2
    cosT = np.ascontiguousarray(cos[:, idx].T).astype(bf)             # [128, S]
    sinT = np.ascontiguousarray(sin[:, idx].T).astype(bf)

    wq = np.asarray(wq, np.float32)
    wk = np.asarray(wk, np.float32)
    wv = np.asarray(wv, np.float32)
    wo = np.asarray(wo, np.float32)

    permT = np.zeros((128, 128), np.float32)
    me = np.arange(0, 128, 2)
    permT[me + 1, me] = -1.0      # swapped[even m] = -raw[m+1]
    permT[me, me + 1] = 1.0       # swapped[odd m]  = +raw[m-1]
    permT = permT.astype(bf)

    pp_ = np.arange(128)
    mm_ = np.arange(64)
    maskI = (-240.0 * ((pp_[:, None] % 64) == mm_[None, :])).astype(np.float32).astype(bf)
    qq_ = np.arange(128)
    stepU = np.stack(
        [(pp_[:, None] % 64) + 64 * s > qq_[None, :] for s in range(2)], axis=1
    ).astype(np.float32).astype(bf)   # [128, 2, 128]

    xTb = [np.ascontiguousarray(x[b].T).astype(bf) for b in range(B)]

    in_maps = []
    for c in range(NCORES):
        b, g = c // 4, c % 4
        rows = slice(g * EL, (g + 1) * EL)
        in_maps.append({
            "xT": xTb[b],
            "cosT": cosT,
            "sinT": sinT,
            "wqT": np.ascontiguousarray(wq[rows, :].T).astype(bf),
            "wkT": np.ascontiguousarray(wk[rows, :].T).astype(bf),
            "permT": permT,
            "wvT": np.ascontiguousarray(wv[rows, :].T).astype(bf),
            "woT": np.ascontiguousarray(wo[:, rows].T).astype(bf),
            "maskI": maskI,
            "stepU": stepU,
        })
    return in_maps


def unshard(results):
    acc = np.zeros((B, D, S), np.float32)
    for c, r in enumerate(results):
        b = c // 4
        acc[b, :, 0:1536] += np.asarray(r["out"], np.float32)[:, 0:1536]
        acc[b, :, 1536:S] += np.asarray(r["o3a"], np.float32)
        acc[b, :, 1536:S] += np.asarray(r["o3b"], np.float32)
    return np.ascontiguousarray(acc.transpose(0, 2, 1))


def kernel(x, token_positions, wq, wk, wv, wo):
    from concourse.bass_utils import run_bass_kernel_spmd

    nc = build_nc(debug=False)
    in_maps = make_in_maps(x, token_positions, wq, wk, wv, wo)
    res = run_bass_kernel_spmd(nc, in_maps, core_ids=list(range(NCORES)))
    return unshard(res.results)


if __name__ == "__main__":
    # smoke test with random data
    rng = np.random.default_rng(0)
    x = rng.standard_normal((B, S, D), dtype=np.float32)
    tp = np.arange(S, dtype=np.int32)
    ws = [rng.standard_normal((D, D), dtype=np.float32) * 0.02 for _ in range(4)]
    out = kernel(x, tp, *ws)
    print(out.shape, out.dtype)
